# revision 19
# baseline (speedup 1.0000x reference)
"""Multi-head attention (B=4, S=2048, D=1024, H=16, Dh=64) on 8 TRN2 NeuronCores.

Sharding: core c -> batch b = c//2, head-group g = c%2 (8 heads, output cols
g*512:(g+1)*512).  Host ships x pre-transposed ([D, S]) and weights in bf16;
each core runs attention for its (batch, 8 heads) slice; host concatenates the
per-core [2048, 512] outputs.

Per-core kernel (bf16 compute, f32 accumulation):
  - qTz = per-head zero-padded q (head rows in its pair slot, zeros in the
    other head's rows): scores run as K=128 matmuls (lhsT = packed kT pair,
    the foreign head's k rows hit q zeros).  K=64 same-row-group matmuls
    serialize on LDWEIGHTS; full-K rotating weights pipeline cleanly.
  - v natural = xT.T @ Wv (+bias via K=1 ones-matmul), augmented with a
    ones-column per head so the AV matmul also produces softmax denominators.
  - per head, per sq-tile: scoresT[sk,sq] K=128 single-shot matmuls into
    double-buffered [128,1024] f32 PSUM; exp on ScalarE (1024 wide, the
    1/sqrt(1024) scale folded into the activation; scores are O(1), no
    max-subtraction needed); out_hT/denoms accumulate in PSUM over sk chunks.
  - PE-transpose [65,128] slabs -> natural [128,64|denom] -> reciprocal +
    per-partition scalar multiply -> out rows.
  - v/qk projections and the previous head's transpose/normalize tail are
    interleaved into the attention chunk stream so the TensorE's slack under
    the ScalarE-bound exp is spent on useful work (and HAM stays warm).
"""

import numpy as np
import ml_dtypes
from contextlib import ExitStack

import concourse.bass as bass
import concourse.bacc as bacc
import concourse.mybir as mybir
import concourse.tile as tile
from concourse.bass_utils import run_bass_kernel_spmd
from concourse.masks import make_identity

F32 = mybir.dt.float32
BF16 = mybir.dt.bfloat16

B, S, D = 4, 2048, 1024
H, DH = 16, 64
N_CORES = 8
HPC = 8          # heads per core
DPC = HPC * DH   # output cols per core = 512
SCALE = 1.0 / 32.0  # 1/sqrt(D)

KD = D // 128    # 8 contraction chunks over d_in
NS = S // 128    # 16 sequence chunks
MB = DPC // 128  # 4 partition blocks (head pairs)
NT = S // 1024   # 2 sq tiles

_CACHE = {}


def _build_program():
    nc = bacc.Bacc("TRN2", target_bir_lowering=False, debug=False)

    xt_ext = nc.dram_tensor("xt", [D, S], BF16, kind="ExternalInput").ap()
    wq_ext = nc.dram_tensor("wq", [D, DPC], BF16, kind="ExternalInput").ap()
    wk_ext = nc.dram_tensor("wk", [D, DPC], BF16, kind="ExternalInput").ap()
    wv_ext = nc.dram_tensor("wv", [D, DPC], BF16, kind="ExternalInput").ap()
    bq_ext = nc.dram_tensor("bq", [DPC], F32, kind="ExternalInput").ap()
    bk_ext = nc.dram_tensor("bk", [DPC], F32, kind="ExternalInput").ap()
    bv_ext = nc.dram_tensor("bv", [DPC], F32, kind="ExternalInput").ap()
    out_ext = nc.dram_tensor("out", [S, DPC], F32, kind="ExternalOutput").ap()

    with tile.TileContext(nc) as tc, ExitStack() as ctx:
        singles = ctx.enter_context(tc.tile_pool(name="singles", bufs=1))

        # --- DMAs: x first, then weights in use order ---
        xT = [singles.tile([128, S], BF16, tag=f"xT{j}", name=f"xT{j}") for j in range(KD)]
        for j in range(KD):
            nc.sync.dma_start(out=xT[j], in_=xt_ext[j * 128:(j + 1) * 128, :])

        w_bf = {}
        for name, ext in (("wq", wq_ext), ("wk", wk_ext), ("wv", wv_ext)):
            tiles = []
            for k in range(KD):
                wb = singles.tile([128, DPC], BF16, tag=f"{name}_bf{k}", name=f"{name}_bf{k}")
                nc.sync.dma_start(out=wb, in_=ext[k * 128:(k + 1) * 128, :])
                tiles.append(wb)
            w_bf[name] = tiles

        identity = singles.tile([128, 128], BF16, tag="identity")
        make_identity(nc, identity)
        ones_row = singles.tile([1, 128], BF16, tag="ones_row")
        nc.vector.memset(ones_row, 1.0)

        bq_col = []
        bk_col = []
        for m in range(MB):
            t = singles.tile([128, 1], F32, tag=f"bq{m}", name=f"bq{m}")
            nc.sync.dma_start(
                out=t, in_=bq_ext[m * 128:(m + 1) * 128].rearrange("(p o) -> p o", o=1)
            )
            bq_col.append(t)
            t = singles.tile([128, 1], F32, tag=f"bk{m}", name=f"bk{m}")
            nc.sync.dma_start(
                out=t, in_=bk_ext[m * 128:(m + 1) * 128].rearrange("(p o) -> p o", o=1)
            )
            bk_col.append(t)
        bv_f32 = singles.tile([1, DPC], F32, tag="bv_f32")
        nc.sync.dma_start(out=bv_f32, in_=bv_ext.rearrange("(o n) -> o n", o=1))
        bv_row = singles.tile([1, DPC], BF16, tag="bv_row")
        nc.vector.tensor_copy(bv_row, bv_f32)

        # --- persistent sbuf tensors ---
        # q/k stored as packed head pairs (A rows 0:64, B rows 64:128) plus
        # half-swapped copies, so score matmuls can alternate PE row groups
        # every sk chunk (same-row-group K=64 matmuls serialize on LDWEIGHTS;
        # alternating groups run two matmuls concurrently)
        qTp = [singles.tile([128, S], BF16, tag=f"qTp{m}", name=f"qTp{m}") for m in range(MB)]
        qTs = [singles.tile([128, S], BF16, tag=f"qTs{m}", name=f"qTs{m}") for m in range(MB)]
        kT = [singles.tile([128, S], BF16, tag=f"kT{m}", name=f"kTt{m}") for m in range(MB)]
        kTs = [singles.tile([128, S], BF16, tag=f"kTs{m}", name=f"kTs{m}") for m in range(MB)]
        vsb = [singles.tile([128, HPC, DH + 1], BF16, tag=f"v{i}", name=f"v{i}") for i in range(NS)]
        out_full = [singles.tile([128, DPC], F32, tag=f"of{i}", name=f"of{i}") for i in range(NS)]

        # --- psum pools: scores 2x[128,1024]f32 (4 banks) + shared
        # accumulator/projection/transpose pool 2x2 banks = 8 banks total ---
        s_psum = ctx.enter_context(tc.tile_pool(name="s_psum", bufs=2, space="PSUM"))
        o_psum = ctx.enter_context(tc.tile_pool(name="o_psum", bufs=2, space="PSUM"))

        e_pool = ctx.enter_context(tc.tile_pool(name="e_pool", bufs=4))
        attn_sb = ctx.enter_context(tc.tile_pool(name="attn_sb", bufs=3))
        ot_sb = ctx.enter_context(tc.tile_pool(name="ot_sb", bufs=8))

        def gen_qk_proj(m, n):
            """Generator: q/k projection group for pair m, 2 matmuls per step."""
            sl = slice(n * 512, (n + 1) * 512)
            ps = o_psum.tile([128, 512], F32, tag="po", name=f"ppq{m}_{n}")
            for k in range(KD):
                nc.tensor.matmul(
                    ps,
                    lhsT=w_bf["wq"][k][:, m * 128:(m + 1) * 128],
                    rhs=xT[k][:, sl],
                    start=(k == 0),
                    stop=(k == KD - 1),
                )
                if k % 2 == 1:
                    yield
            nc.vector.tensor_scalar_add(qTp[m][:, sl], ps, bq_col[m])
            ps = o_psum.tile([128, 512], F32, tag="po", name=f"ppk{m}_{n}")
            for k in range(KD):
                nc.tensor.matmul(
                    ps,
                    lhsT=w_bf["wk"][k][:, m * 128:(m + 1) * 128],
                    rhs=xT[k][:, sl],
                    start=(k == 0),
                    stop=(k == KD - 1),
                )
                if k % 2 == 1:
                    yield
            nc.vector.tensor_scalar_add(kT[m][:, sl], ps, bk_col[m])

        def emit_qk_proj(m, n):
            for _ in gen_qk_proj(m, n):
                pass

        def emit_swaps(m):
            """Build the half-swapped q/k copies for pair m (cross-partition
            moves need DMA; DVE lanes are hardwired)."""
            nc.sync.dma_start(out=qTs[m][64:128, :], in_=qTp[m][0:64, :])
            nc.sync.dma_start(out=qTs[m][0:64, :], in_=qTp[m][64:128, :])
            nc.sync.dma_start(out=kTs[m][64:128, :], in_=kT[m][0:64, :])
            nc.sync.dma_start(out=kTs[m][0:64, :], in_=kT[m][64:128, :])

        def emit_v_proj(i):
            ps = o_psum.tile([128, 512], F32, tag="po", name=f"vp{i}")
            for k in range(KD):
                nc.tensor.matmul(
                    ps,
                    lhsT=xT[k][:, i * 128:(i + 1) * 128],
                    rhs=w_bf["wv"][k],
                    start=(k == 0),
                    stop=False,
                )
            nc.tensor.matmul(ps, lhsT=ones_row, rhs=bv_row, start=False, stop=True)
            nc.vector.tensor_copy(
                vsb[i][:, :, 0:DH], ps.rearrange("p (h d) -> p h d", h=HPC)
            )
            nc.vector.memset(vsb[i][:, :, DH:DH + 1], 1.0)

        def emit_head_tail_piece(h, o_sb, c2):
            """Transpose + normalize + write one 128-row slab of head h."""
            pt = o_psum.tile([128, 65], BF16, tag="po", name=f"pt{h}_{c2}")
            nc.tensor.transpose(
                pt, o_sb[:, c2 * 128:(c2 + 1) * 128], identity[0:65, 0:65]
            )
            ot = ot_sb.tile([128, 65], BF16, tag="ot", name=f"ot{h}_{c2}")
            nc.vector.tensor_copy(ot, pt)
            rc = ot_sb.tile([128, 1], F32, tag="rc", name=f"rc{h}_{c2}")
            nc.vector.reciprocal(rc, ot[:, DH:DH + 1])
            nc.vector.tensor_scalar_mul(
                out_full[c2][:, h * DH:(h + 1) * DH], ot[:, 0:DH], rc
            )

        # warm the PE clock (HAM) while DMA streams in: each pulse reads the
        # just-arrived xT chunk so PE activity spans the whole load window
        warm = o_psum.tile([128, 512], F32, tag="po", name="warm")
        for j in range(KD):
            for i in range(5):
                nc.tensor.matmul(
                    warm, lhsT=identity, rhs=xT[j][:, 0:512], start=True, stop=True
                )

        emit_qk_proj(0, 0)
        emit_qk_proj(0, 1)
        emit_qk_proj(0, 2)
        emit_qk_proj(0, 3)
        emit_swaps(0)

        o_sbs = {}
        for h in range(HPC):
            hp = h // 2
            o_sb = attn_sb.tile([65, S], BF16, tag="o_sb", name=f"osb{h}")
            o_sbs[h] = o_sb
            # interleaved filler for this head's 32 chunk iterations:
            # each item is a small closure emitting a couple of PE ops
            filler = []
            if h > 0:
                filler += [
                    (emit_head_tail_piece, (h - 1, o_sbs[h - 1], c2)) for c2 in range(NS)
                ]
                if h % 2 == 1 and hp + 1 < MB:
                    for n in range(4):
                        g = gen_qk_proj(hp + 1, n)
                        filler += [(g.__next__, ())] * 8 + [
                            (lambda gg=g: list(gg), ())
                        ]
                    filler.append((emit_swaps, (hp + 1,)))
            fi = 0
            n_iters = NT * NS
            for t in range(NT):
                po = o_psum.tile([128, 1024], F32, tag="po", name=f"po{h}_{t}")
                for c in range(NS):
                    it = t * NS + c
                    if filler:
                        # drain filler by ~3/4 through the unit so the next
                        # pair's kT/qTz are ready before its first scores
                        want = min(((it + 1) * len(filler)) // (n_iters - 8), len(filler))
                        while fi < want:
                            fn, args = filler[fi]
                            try:
                                fn(*args)
                            except StopIteration:
                                pass
                            fi += 1
                    psc = s_psum.tile([128, 1024], F32, tag="psc", name=f"ps{h}_{t}_{c}")
                    # alternate PE row groups across sk chunks
                    if c % 2 == 0:
                        kk, qq = kT[hp], qTp[hp]
                        r = 64 * (h % 2)
                    else:
                        kk, qq = kTs[hp], qTs[hp]
                        r = 64 * (1 - (h % 2))
                    for half in range(2):
                        nc.tensor.matmul(
                            psc[:, half * 512:(half + 1) * 512],
                            lhsT=kk[r:r + 64, c * 128:(c + 1) * 128],
                            rhs=qq[
                                r:r + 64,
                                t * 1024 + half * 512:t * 1024 + (half + 1) * 512,
                            ],
                            start=True, stop=True,
                        )
                    e = e_pool.tile([128, 1024], BF16, tag="e", name=f"e{h}_{t}_{c}")
                    nc.scalar.activation(
                        e, psc, mybir.ActivationFunctionType.Exp, scale=SCALE
                    )
                    if h == 0 and t == 0:
                        # just-in-time v projection, after scores so ScalarE
                        # isn't gated on it; AV below consumes it
                        emit_v_proj(c)
                    for half in range(2):
                        nc.tensor.matmul(
                            po[0:65, half * 512:(half + 1) * 512],
                            lhsT=vsb[c][:, h, :],
                            rhs=e[:, half * 512:(half + 1) * 512],
                            start=(c == 0), stop=(c == NS - 1),
                        )
                nc.vector.tensor_copy(o_sb[0:65, t * 1024:(t + 1) * 1024], po[0:65, :])
            while fi < len(filler):
                fn, args = filler[fi]
                try:
                    fn(*args)
                except StopIteration:
                    pass
                fi += 1
        # tail of the last head
        for c2 in range(NS):
            emit_head_tail_piece(HPC - 1, o_sbs[HPC - 1], c2)

        for i in range(NS):
            nc.sync.dma_start(out=out_ext[i * 128:(i + 1) * 128, :], in_=out_full[i])

    nc.compile()
    return nc


def _get_program():
    if "nc" not in _CACHE:
        _CACHE["nc"] = _build_program()
    return _CACHE["nc"]


def kernel(x, Wq, bq, Wk, bk, Wv, bv, _trace=False):
    bf = ml_dtypes.bfloat16
    x = np.asarray(x, dtype=np.float32)
    Wq = np.asarray(Wq, dtype=np.float32)
    Wk = np.asarray(Wk, dtype=np.float32)
    Wv = np.asarray(Wv, dtype=np.float32)
    bq = np.ascontiguousarray(np.asarray(bq, dtype=np.float32))
    bk = np.ascontiguousarray(np.asarray(bk, dtype=np.float32))
    bv = np.ascontiguousarray(np.asarray(bv, dtype=np.float32))

    nc = _get_program()

    in_maps = []
    for c in range(N_CORES):
        b, g = c // 2, c % 2
        cols = slice(g * DPC, (g + 1) * DPC)
        in_maps.append(
            {
                "xt": np.ascontiguousarray(x[b].T.astype(bf)),
                "wq": np.ascontiguousarray(Wq[:, cols].astype(bf)),
                "wk": np.ascontiguousarray(Wk[:, cols].astype(bf)),
                "wv": np.ascontiguousarray(Wv[:, cols].astype(bf)),
                "bq": np.ascontiguousarray(bq[cols]),
                "bk": np.ascontiguousarray(bk[cols]),
                "bv": np.ascontiguousarray(bv[cols]),
            }
        )

    res = run_bass_kernel_spmd(nc, in_maps, core_ids=list(range(N_CORES)), trace=_trace)
    _CACHE["last_results"] = res

    out = np.empty((B, S, D), dtype=np.float32)
    for c in range(N_CORES):
        b, g = c // 2, c % 2
        out[b, :, g * DPC:(g + 1) * DPC] = res.results[c]["out"]
    return out


# revision 20
# speedup vs baseline: 1.0148x; 1.0148x over previous
"""Multi-head attention (B=4, S=2048, D=1024, H=16, Dh=64) on 8 TRN2 NeuronCores.

Sharding: core c -> batch b = c//2, head-group g = c%2 (8 heads, output cols
g*512:(g+1)*512).  Host ships x pre-transposed ([D, S]) and weights in bf16;
each core runs attention for its (batch, 8 heads) slice; host concatenates the
per-core [2048, 512] outputs.

Per-core kernel (bf16 compute, f32 accumulation):
  - qTz = per-head zero-padded q (head rows in its pair slot, zeros in the
    other head's rows): scores run as K=128 matmuls (lhsT = packed kT pair,
    the foreign head's k rows hit q zeros).  K=64 same-row-group matmuls
    serialize on LDWEIGHTS; full-K rotating weights pipeline cleanly.
  - v natural = xT.T @ Wv (+bias via K=1 ones-matmul), augmented with a
    ones-column per head so the AV matmul also produces softmax denominators.
  - per head, per sq-tile: scoresT[sk,sq] K=128 single-shot matmuls into
    double-buffered [128,1024] f32 PSUM; exp on ScalarE (1024 wide, the
    1/sqrt(1024) scale folded into the activation; scores are O(1), no
    max-subtraction needed); out_hT/denoms accumulate in PSUM over sk chunks.
  - PE-transpose [65,128] slabs -> natural [128,64|denom] -> reciprocal +
    per-partition scalar multiply -> out rows.
  - v/qk projections and the previous head's transpose/normalize tail are
    interleaved into the attention chunk stream so the TensorE's slack under
    the ScalarE-bound exp is spent on useful work (and HAM stays warm).
"""

import numpy as np
import ml_dtypes
from contextlib import ExitStack

import concourse.bass as bass
import concourse.bacc as bacc
import concourse.mybir as mybir
import concourse.tile as tile
from concourse.bass_utils import run_bass_kernel_spmd
from concourse.masks import make_identity

F32 = mybir.dt.float32
BF16 = mybir.dt.bfloat16

B, S, D = 4, 2048, 1024
H, DH = 16, 64
N_CORES = 8
HPC = 8          # heads per core
DPC = HPC * DH   # output cols per core = 512
SCALE = 1.0 / 32.0  # 1/sqrt(D)

KD = D // 128    # 8 contraction chunks over d_in
NS = S // 128    # 16 sequence chunks
MB = DPC // 128  # 4 partition blocks (head pairs)
NT = S // 1024   # 2 sq tiles

_CACHE = {}


def _build_program():
    nc = bacc.Bacc("TRN2", target_bir_lowering=False, debug=False)

    xt_ext = nc.dram_tensor("xt", [D, S], BF16, kind="ExternalInput").ap()
    wq_ext = nc.dram_tensor("wq", [D, DPC], BF16, kind="ExternalInput").ap()
    wk_ext = nc.dram_tensor("wk", [D, DPC], BF16, kind="ExternalInput").ap()
    wv_ext = nc.dram_tensor("wv", [D, DPC], BF16, kind="ExternalInput").ap()
    bq_ext = nc.dram_tensor("bq", [DPC], F32, kind="ExternalInput").ap()
    bk_ext = nc.dram_tensor("bk", [DPC], F32, kind="ExternalInput").ap()
    bv_ext = nc.dram_tensor("bv", [DPC], F32, kind="ExternalInput").ap()
    out_ext = nc.dram_tensor("out", [S, DPC], F32, kind="ExternalOutput").ap()

    with tile.TileContext(nc) as tc, ExitStack() as ctx:
        singles = ctx.enter_context(tc.tile_pool(name="singles", bufs=1))

        # --- DMAs: x first, then weights in use order ---
        xT = [singles.tile([128, S], BF16, tag=f"xT{j}", name=f"xT{j}") for j in range(KD)]
        for j in range(KD):
            nc.sync.dma_start(out=xT[j], in_=xt_ext[j * 128:(j + 1) * 128, :])

        w_bf = {}
        for name, ext in (("wq", wq_ext), ("wk", wk_ext), ("wv", wv_ext)):
            tiles = []
            for k in range(KD):
                wb = singles.tile([128, DPC], BF16, tag=f"{name}_bf{k}", name=f"{name}_bf{k}")
                nc.sync.dma_start(out=wb, in_=ext[k * 128:(k + 1) * 128, :])
                tiles.append(wb)
            w_bf[name] = tiles

        identity = singles.tile([128, 128], BF16, tag="identity")
        make_identity(nc, identity)
        ones_row = singles.tile([1, 128], BF16, tag="ones_row")
        nc.vector.memset(ones_row, 1.0)

        bq_col = []
        bk_col = []
        for m in range(MB):
            t = singles.tile([128, 1], F32, tag=f"bq{m}", name=f"bq{m}")
            nc.sync.dma_start(
                out=t, in_=bq_ext[m * 128:(m + 1) * 128].rearrange("(p o) -> p o", o=1)
            )
            bq_col.append(t)
            t = singles.tile([128, 1], F32, tag=f"bk{m}", name=f"bk{m}")
            nc.sync.dma_start(
                out=t, in_=bk_ext[m * 128:(m + 1) * 128].rearrange("(p o) -> p o", o=1)
            )
            bk_col.append(t)
        bv_f32 = singles.tile([1, DPC], F32, tag="bv_f32")
        nc.sync.dma_start(out=bv_f32, in_=bv_ext.rearrange("(o n) -> o n", o=1))
        bv_row = singles.tile([1, DPC], BF16, tag="bv_row")
        nc.vector.tensor_copy(bv_row, bv_f32)

        # --- persistent sbuf tensors ---
        # q/k stored as packed head pairs (A rows 0:64, B rows 64:128) plus
        # half-swapped copies, so score matmuls can alternate PE row groups
        # every sk chunk (same-row-group K=64 matmuls serialize on LDWEIGHTS;
        # alternating groups run two matmuls concurrently)
        qTp = [singles.tile([128, S], BF16, tag=f"qTp{m}", name=f"qTp{m}") for m in range(MB)]
        qTs = [singles.tile([128, S], BF16, tag=f"qTs{m}", name=f"qTs{m}") for m in range(MB)]
        kT = [singles.tile([128, S], BF16, tag=f"kT{m}", name=f"kTt{m}") for m in range(MB)]
        kTs = [singles.tile([128, S], BF16, tag=f"kTs{m}", name=f"kTs{m}") for m in range(MB)]
        vsb = [singles.tile([128, HPC, DH + 1], BF16, tag=f"v{i}", name=f"v{i}") for i in range(NS)]
        out_full = [singles.tile([128, DPC], F32, tag=f"of{i}", name=f"of{i}") for i in range(NS)]

        # --- psum pools: scores 2x[128,1024]f32 (4 banks) + shared
        # accumulator/projection/transpose pool 2x2 banks = 8 banks total ---
        s_psum = ctx.enter_context(tc.tile_pool(name="s_psum", bufs=2, space="PSUM"))
        o_psum = ctx.enter_context(tc.tile_pool(name="o_psum", bufs=2, space="PSUM"))

        e_pool = ctx.enter_context(tc.tile_pool(name="e_pool", bufs=4))
        attn_sb = ctx.enter_context(tc.tile_pool(name="attn_sb", bufs=3))
        ot_sb = ctx.enter_context(tc.tile_pool(name="ot_sb", bufs=8))

        def gen_qk_proj(m, n):
            """Generator: q/k projection group for pair m, 2 matmuls per step."""
            sl = slice(n * 512, (n + 1) * 512)
            ps = o_psum.tile([128, 512], F32, tag="po", name=f"ppq{m}_{n}")
            for k in range(KD):
                nc.tensor.matmul(
                    ps,
                    lhsT=w_bf["wq"][k][:, m * 128:(m + 1) * 128],
                    rhs=xT[k][:, sl],
                    start=(k == 0),
                    stop=(k == KD - 1),
                )
                if k % 2 == 1:
                    yield
            nc.vector.tensor_scalar_add(qTp[m][:, sl], ps, bq_col[m])
            ps = o_psum.tile([128, 512], F32, tag="po", name=f"ppk{m}_{n}")
            for k in range(KD):
                nc.tensor.matmul(
                    ps,
                    lhsT=w_bf["wk"][k][:, m * 128:(m + 1) * 128],
                    rhs=xT[k][:, sl],
                    start=(k == 0),
                    stop=(k == KD - 1),
                )
                if k % 2 == 1:
                    yield
            nc.vector.tensor_scalar_add(kT[m][:, sl], ps, bk_col[m])

        def emit_qk_proj(m, n):
            for _ in gen_qk_proj(m, n):
                pass

        def emit_swaps(m):
            """Build the half-swapped q/k copies for pair m (cross-partition
            moves need DMA; DVE lanes are hardwired)."""
            nc.sync.dma_start(out=qTs[m][64:128, :], in_=qTp[m][0:64, :])
            nc.sync.dma_start(out=qTs[m][0:64, :], in_=qTp[m][64:128, :])
            nc.sync.dma_start(out=kTs[m][64:128, :], in_=kT[m][0:64, :])
            nc.sync.dma_start(out=kTs[m][0:64, :], in_=kT[m][64:128, :])

        def emit_v_proj(i):
            ps = o_psum.tile([128, 512], F32, tag="po", name=f"vp{i}")
            for k in range(KD):
                nc.tensor.matmul(
                    ps,
                    lhsT=xT[k][:, i * 128:(i + 1) * 128],
                    rhs=w_bf["wv"][k],
                    start=(k == 0),
                    stop=False,
                )
            nc.tensor.matmul(ps, lhsT=ones_row, rhs=bv_row, start=False, stop=True)
            nc.vector.tensor_copy(
                vsb[i][:, :, 0:DH], ps.rearrange("p (h d) -> p h d", h=HPC)
            )
            nc.vector.memset(vsb[i][:, :, DH:DH + 1], 1.0)

        def emit_head_tail_piece(h, o_sb, c2):
            """Transpose + normalize + write one 128-row slab of head h."""
            pt = o_psum.tile([128, 65], BF16, tag="po", name=f"pt{h}_{c2}")
            nc.tensor.transpose(
                pt, o_sb[:, c2 * 128:(c2 + 1) * 128], identity[0:65, 0:65]
            )
            ot = ot_sb.tile([128, 65], BF16, tag="ot", name=f"ot{h}_{c2}")
            nc.vector.tensor_copy(ot, pt)
            rc = ot_sb.tile([128, 1], F32, tag="rc", name=f"rc{h}_{c2}")
            nc.vector.reciprocal(rc, ot[:, DH:DH + 1])
            nc.vector.tensor_scalar_mul(
                out_full[c2][:, h * DH:(h + 1) * DH], ot[:, 0:DH], rc
            )

        # warm the PE clock (HAM) while DMA streams in: each pulse reads the
        # just-arrived xT chunk so PE activity spans the whole load window
        warm = o_psum.tile([128, 512], F32, tag="po", name="warm")
        for j in range(KD):
            for i in range(5):
                nc.tensor.matmul(
                    warm, lhsT=identity, rhs=xT[j][:, 0:512], start=True, stop=True
                )

        emit_qk_proj(0, 0)
        emit_qk_proj(0, 1)
        emit_qk_proj(0, 2)
        emit_qk_proj(0, 3)
        emit_swaps(0)

        o_sbs = {}
        for h in range(HPC):
            hp = h // 2
            o_sb = attn_sb.tile([65, S], BF16, tag="o_sb", name=f"osb{h}")
            o_sbs[h] = o_sb
            # interleaved filler for this head's 32 chunk iterations:
            # each item is a small closure emitting a couple of PE ops
            filler = []
            if h > 0:
                filler += [
                    (emit_head_tail_piece, (h - 1, o_sbs[h - 1], c2)) for c2 in range(NS)
                ]
                if h % 2 == 1 and hp + 1 < MB:
                    for n in range(4):
                        g = gen_qk_proj(hp + 1, n)
                        filler += [(g.__next__, ())] * 8 + [
                            (lambda gg=g: list(gg), ())
                        ]
                    filler.append((emit_swaps, (hp + 1,)))
            fi = 0
            n_iters = NT * NS
            for t in range(NT):
                po = o_psum.tile([128, 1024], F32, tag="po", name=f"po{h}_{t}")
                for c0 in range(0, NS, 2):
                    it = t * NS + c0
                    if filler:
                        # drain filler by ~3/4 through the unit so the next
                        # pair's q/k are ready before its first scores
                        want = min(((it + 2) * len(filler)) // (n_iters - 8), len(filler))
                        while fi < want:
                            fn, args = filler[fi]
                            try:
                                fn(*args)
                            except StopIteration:
                                pass
                            fi += 1
                    # 4 score matmuls: row groups grp,grp,grp',grp' -- the two
                    # K=64 pairs run concurrently on the PE's row halves
                    pscs = []
                    es = []
                    for c in (c0, c0 + 1):
                        psc = s_psum.tile([128, 1024], F32, tag="psc", name=f"ps{h}_{t}_{c}")
                        pscs.append(psc)
                        if c % 2 == 0:
                            kk, qq = kT[hp], qTp[hp]
                            r = 64 * (h % 2)
                        else:
                            kk, qq = kTs[hp], qTs[hp]
                            r = 64 * (1 - (h % 2))
                        for half in range(2):
                            nc.tensor.matmul(
                                psc[:, half * 512:(half + 1) * 512],
                                lhsT=kk[r:r + 64, c * 128:(c + 1) * 128],
                                rhs=qq[
                                    r:r + 64,
                                    t * 1024 + half * 512:t * 1024 + (half + 1) * 512,
                                ],
                                start=True, stop=True,
                            )
                    for i, c in enumerate((c0, c0 + 1)):
                        e = e_pool.tile([128, 1024], BF16, tag="e", name=f"e{h}_{t}_{c}")
                        es.append(e)
                        nc.scalar.activation(
                            e, pscs[i], mybir.ActivationFunctionType.Exp, scale=SCALE
                        )
                    if h == 0 and t == 0:
                        emit_v_proj(c0)
                        emit_v_proj(c0 + 1)
                    # 4 AV matmuls, K=128 rotating weights -> LDW pipelines
                    for i, c in enumerate((c0, c0 + 1)):
                        for half in range(2):
                            nc.tensor.matmul(
                                po[0:65, half * 512:(half + 1) * 512],
                                lhsT=vsb[c][:, h, :],
                                rhs=es[i][:, half * 512:(half + 1) * 512],
                                start=(c == 0), stop=(c == NS - 1),
                            )
                nc.vector.tensor_copy(o_sb[0:65, t * 1024:(t + 1) * 1024], po[0:65, :])
            while fi < len(filler):
                fn, args = filler[fi]
                try:
                    fn(*args)
                except StopIteration:
                    pass
                fi += 1
        # tail of the last head
        for c2 in range(NS):
            emit_head_tail_piece(HPC - 1, o_sbs[HPC - 1], c2)

        for i in range(NS):
            nc.sync.dma_start(out=out_ext[i * 128:(i + 1) * 128, :], in_=out_full[i])

    nc.compile()
    return nc


def _get_program():
    if "nc" not in _CACHE:
        _CACHE["nc"] = _build_program()
    return _CACHE["nc"]


def kernel(x, Wq, bq, Wk, bk, Wv, bv, _trace=False):
    bf = ml_dtypes.bfloat16
    x = np.asarray(x, dtype=np.float32)
    Wq = np.asarray(Wq, dtype=np.float32)
    Wk = np.asarray(Wk, dtype=np.float32)
    Wv = np.asarray(Wv, dtype=np.float32)
    bq = np.ascontiguousarray(np.asarray(bq, dtype=np.float32))
    bk = np.ascontiguousarray(np.asarray(bk, dtype=np.float32))
    bv = np.ascontiguousarray(np.asarray(bv, dtype=np.float32))

    nc = _get_program()

    in_maps = []
    for c in range(N_CORES):
        b, g = c // 2, c % 2
        cols = slice(g * DPC, (g + 1) * DPC)
        in_maps.append(
            {
                "xt": np.ascontiguousarray(x[b].T.astype(bf)),
                "wq": np.ascontiguousarray(Wq[:, cols].astype(bf)),
                "wk": np.ascontiguousarray(Wk[:, cols].astype(bf)),
                "wv": np.ascontiguousarray(Wv[:, cols].astype(bf)),
                "bq": np.ascontiguousarray(bq[cols]),
                "bk": np.ascontiguousarray(bk[cols]),
                "bv": np.ascontiguousarray(bv[cols]),
            }
        )

    res = run_bass_kernel_spmd(nc, in_maps, core_ids=list(range(N_CORES)), trace=_trace)
    _CACHE["last_results"] = res

    out = np.empty((B, S, D), dtype=np.float32)
    for c in range(N_CORES):
        b, g = c // 2, c % 2
        out[b, :, g * DPC:(g + 1) * DPC] = res.results[c]["out"]
    return out
